# revision 7
# baseline (speedup 1.0000x reference)
"""Trainium2 Bass kernel for nn_GATLayer (2x relational attention, B=8,N=2048,D=256).

Key math: the score Linear(2d->1) on concat decomposes additively, so
score[b,i,j] = qdot[b,i] + kdot[b,j] + bs.  Softmax over j is invariant to
per-row constants, hence attn[b,i,:] = softmax_j(kdot[b,:]) for EVERY i.
The whole attention collapses to per-batch vector work:

  layer(p_in, x_in, mask):                       # kv side = x_in
    e    = exp(x_in @ u) * mask                  # u = Wk @ Ws[d:,0]
    A    = sum(e);  xbar = (e @ x_in) / A
    ctx  = xbar @ Wv + bv                        # (d,) per batch
    g    = sigmoid(p_in @ w + ctx.wg1 + bg)      # w = Wg[:d,0]+Wg[d:,0]
    out  = p_in + g * ctx

  x_new = 2x + g1*ctx1          (layer1: p_in=x, kv=p, no mask)
  p_new = 2p + g2*ctx2          (layer2: p_in=p, kv=x_new, mask)

Layer-2 terms are re-expressed against the ORIGINAL x (never materializing
x_new on the critical path):
  kdot2 = x_new@u2 = 2(x@u2) + (ctx1.u2)*g1
  e2@x_new = 2(e2@x) + (e2.g1)*ctx1

Sharding: data-parallel over batch, one batch per NeuronCore (8 cores).
"""

import numpy as np

B, N, D = 8, 2048, 256
P = 128            # partitions
T = N // P         # 16 tiles of (128, 256) per tensor
NCORES = 8
CHUNK = 4          # DMA / cast granularity in tiles
NS = 10            # tiles 0..NS-1 use the PE path for the output combine,
                   # tiles NS..T-1 use the ACT+DVE path (engine balancing)


def _fold_host(inputs):
    """Fold weights on host (fp64 for accuracy, cast to f32/bf16)."""
    import ml_dtypes

    f = {}
    for L in ("ra1", "ra2"):
        Wk = inputs[f"{L}_Wk"].astype(np.float64)
        Ws = inputs[f"{L}_Ws"].astype(np.float64)
        Wg = inputs[f"{L}_Wg"].astype(np.float64)
        u = Wk @ Ws[D:, 0]                       # (D,)
        w = Wg[:D, 0] + Wg[D:, 0]                # (D,)
        f[f"{L}_u"] = u.astype(np.float32)
        f[f"{L}_w"] = w.astype(np.float32)
        f[f"{L}_wg1"] = Wg[:D, 0].astype(np.float32)
        f[f"{L}_bv"] = inputs[f"{L}_bv"].astype(np.float32)
        f[f"{L}_bg"] = float(inputs[f"{L}_bg"][0])
        f[f"{L}_Wv_bf"] = inputs[f"{L}_Wv"].astype(ml_dtypes.bfloat16)
    return f


def _perm(a):
    # (2048, 256) -> (128, 16*256): partition p holds rows {p, 128+p, ...}
    return np.ascontiguousarray(
        a.reshape(T, P, D).transpose(1, 0, 2).reshape(P, T * D))


def _unperm(a):
    return np.ascontiguousarray(
        a.reshape(P, T, D).transpose(1, 0, 2).reshape(N, D))


def build(inputs):
    """Build the Bass program + per-core input maps.

    Returns (nc, in_maps, post) where post(results) -> (x_new, p_new).
    """
    import ml_dtypes
    import concourse.bacc as bacc
    import concourse.tile as tile
    import concourse.mybir as mybir

    f32 = mybir.dt.float32
    bf16 = mybir.dt.bfloat16
    MUL = mybir.AluOpType.mult
    ADD = mybir.AluOpType.add
    EXP = mybir.ActivationFunctionType.Exp
    SIG = mybir.ActivationFunctionType.Sigmoid
    CPY = mybir.ActivationFunctionType.Copy

    fold = _fold_host(inputs)
    bg1, bg2 = fold["ra1_bg"], fold["ra2_bg"]

    nc = bacc.Bacc()

    # ---- DRAM I/O -------------------------------------------------------
    x_d = nc.dram_tensor("x", [P, T * D], f32, kind="ExternalInput")
    p_d = nc.dram_tensor("p", [P, T * D], f32, kind="ExternalInput")
    m_d = nc.dram_tensor("mask", [P, T], f32, kind="ExternalInput")
    wv1_d = nc.dram_tensor("wv1", [P, 2 * D], bf16, kind="ExternalInput")
    wv2_d = nc.dram_tensor("wv2", [P, 2 * D], bf16, kind="ExternalInput")
    eye_d = nc.dram_tensor("eye", [P, P], f32, kind="ExternalInput")
    twoi_d = nc.dram_tensor("two_i", [P, P], f32, kind="ExternalInput")
    onesrf_d = nc.dram_tensor("ones_r_f", [1, P], f32, kind="ExternalInput")
    onesrb_d = nc.dram_tensor("ones_r_b", [1, P], bf16, kind="ExternalInput")
    onescf_d = nc.dram_tensor("ones_c_f", [P, 1], f32, kind="ExternalInput")
    bgs_d = nc.dram_tensor("bgs", [1, 2], f32, kind="ExternalInput")
    # bf16 rows broadcast on device: u1, 2*u2, w1, w2
    rowsb_d = nc.dram_tensor("rows_b", [1, 4 * D], bf16, kind="ExternalInput")
    # f32 rows used directly: u2, wg11, wg12, bv1, bv2
    rowsf_d = nc.dram_tensor("rows_f", [1, 5 * D], f32, kind="ExternalInput")

    xo_d = nc.dram_tensor("x_out", [P, T * D], f32, kind="ExternalOutput")
    po_d = nc.dram_tensor("p_out", [P, T * D], f32, kind="ExternalOutput")

    with tile.TileContext(nc) as tc:
        with (
            tc.tile_pool(name="big", bufs=1) as big,
            tc.tile_pool(name="small", bufs=1) as small,
            tc.tile_pool(name="ps_g", bufs=3, space="PSUM") as ps_g,
            tc.tile_pool(name="ps_xb", bufs=2, space="PSUM") as ps_xb,
            tc.tile_pool(name="ps_sm", bufs=3, space="PSUM") as ps_sm,
        ):
            # ---- persistent SBUF ----------------------------------------
            x_sb = big.tile([P, T, D], f32)
            p_sb = big.tile([P, T, D], f32)
            x_bf = big.tile([P, T, D], bf16)
            p_bf = big.tile([P, T, D], bf16)
            xn_sb = big.tile([P, T, D], f32)
            pn_sb = big.tile([P, T, D], f32)
            wv1 = big.tile([P, 2, D], bf16)
            wv2 = big.tile([P, 2, D], bf16)
            eye = big.tile([P, P], f32)
            twoi = big.tile([P, P], f32)
            ones_rf = small.tile([1, P], f32)
            ones_rb = small.tile([1, P], bf16)
            ones_cf = small.tile([P, 1], f32)
            rows_b = small.tile([1, 4, D], bf16)
            rows_f = small.tile([1, 5, D], f32)
            mask_sb = small.tile([P, T], f32)
            bgs = small.tile([1, 2], f32)

            # ---- loads --------------------------------------------------
            for ch in range(0, T, CHUNK):
                s = slice(ch * D, (ch + CHUNK) * D)
                nc.sync.dma_start(x_sb[:, ch:ch + CHUNK, :], x_d[:, s])
                nc.sync.dma_start(p_sb[:, ch:ch + CHUNK, :], p_d[:, s])
            nc.sync.dma_start(mask_sb[:], m_d[:])
            nc.sync.dma_start(wv1[:], wv1_d[:])
            nc.sync.dma_start(wv2[:], wv2_d[:])
            nc.sync.dma_start(eye[:], eye_d[:])
            nc.sync.dma_start(twoi[:], twoi_d[:])
            nc.sync.dma_start(ones_rf[:], onesrf_d[:])
            nc.sync.dma_start(ones_rb[:], onesrb_d[:])
            nc.sync.dma_start(ones_cf[:], onescf_d[:])
            nc.sync.dma_start(rows_b[:], rowsb_d[:])
            nc.sync.dma_start(rows_f[:], rowsf_d[:])
            nc.sync.dma_start(bgs[:], bgs_d[:])

            u2_row = rows_f[:, 0, :]
            wg11_row = rows_f[:, 1, :]
            wg12_row = rows_f[:, 2, :]
            bv1_row = rows_f[:, 3, :]
            bv2_row = rows_f[:, 4, :]

            # ---- bf16 casts (DVE for x, ACT for p) ----------------------
            for ch in range(0, T, CHUNK):
                nc.vector.tensor_copy(x_bf[:, ch:ch + CHUNK, :],
                                      x_sb[:, ch:ch + CHUNK, :])
                nc.scalar.copy(p_bf[:, ch:ch + CHUNK, :],
                               p_sb[:, ch:ch + CHUNK, :])

            # ---- broadcast the 4 bf16 weight rows to 128 partitions -----
            wbc = big.tile([P, 4, D], bf16)   # u1b, u2b2, w1b, w2b
            for i in range(4):
                bc_ps = ps_sm.tile([P, D], f32, tag="sm")
                nc.tensor.matmul(bc_ps[:], ones_rb[:], rows_b[:, i, :],
                                 start=True, stop=True)
                nc.scalar.copy(wbc[:, i, :], bc_ps[:])

            # ---- the 4 row-dot passes (DVE, bf16, fused mult+reduce) ----
            sk1 = small.tile([P, T], f32)
            gx1 = small.tile([P, T], f32)
            sx2 = small.tile([P, T], f32)
            gp2 = small.tile([P, T], f32)
            junk = big.tile([P, D], bf16)
            for t in range(T):
                nc.vector.scalar_tensor_tensor(
                    out=junk[:], in0=p_bf[:, t, :], scalar=1.0,
                    in1=wbc[:, 0, :], op0=MUL, op1=MUL,
                    accum_out=sk1[:, t:t + 1])
                nc.vector.scalar_tensor_tensor(
                    out=junk[:], in0=x_bf[:, t, :], scalar=1.0,
                    in1=wbc[:, 1, :], op0=MUL, op1=MUL,
                    accum_out=sx2[:, t:t + 1])
                nc.vector.scalar_tensor_tensor(
                    out=junk[:], in0=x_bf[:, t, :], scalar=1.0,
                    in1=wbc[:, 2, :], op0=MUL, op1=MUL,
                    accum_out=gx1[:, t:t + 1])
                nc.vector.scalar_tensor_tensor(
                    out=junk[:], in0=p_bf[:, t, :], scalar=1.0,
                    in1=wbc[:, 3, :], op0=MUL, op1=MUL,
                    accum_out=gp2[:, t:t + 1])

            # =============== layer 1 attention (kv = p) ==================
            e1f = small.tile([P, T], f32)
            e1b = small.tile([P, T], bf16)
            nc.scalar.activation(e1f[:], sk1[:], EXP)
            nc.vector.tensor_copy(e1b[:], e1f[:])

            # A1 = sum(e1); r1 = 1/A1
            a1_ps = ps_sm.tile([1, T], f32, tag="sm")
            nc.tensor.matmul(a1_ps[:], ones_cf[:], e1f[:], start=True, stop=True)
            a1 = small.tile([1, 1], f32, tag="a1")
            nc.vector.tensor_reduce(a1[:], a1_ps[:], axis=mybir.AxisListType.X,
                                    op=ADD)
            r1 = small.tile([1, 1], f32, tag="r1")
            nc.vector.reciprocal(r1[:], a1[:])

            # xbarT1[d,c] = sum_j e1[j] * p[j, d]  (unnormalized)
            xb1_ps = ps_xb.tile([P, 2], f32, tag="xb")
            for c in range(2):
                for t in range(T):
                    nc.tensor.matmul(
                        xb1_ps[:, c:c + 1],
                        p_bf[:, t, c * P:(c + 1) * P],
                        e1b[:, t:t + 1],
                        start=(t == 0), stop=(t == T - 1))
            xb1 = small.tile([P, 2], bf16, tag="xb1s")
            nc.vector.tensor_copy(xb1[:], xb1_ps[:])

            # ctx1 = xbar1 @ Wv1 / A1 + bv1
            c1_ps = ps_sm.tile([1, D], f32, tag="sm")
            for c in range(2):
                nc.tensor.matmul(c1_ps[:], xb1[:, c:c + 1], wv1[:, c, :],
                                 start=(c == 0), stop=(c == 1))
            ctx1 = small.tile([1, D], f32, tag="ctx1")
            nc.vector.scalar_tensor_tensor(
                out=ctx1[:], in0=c1_ps[:], scalar=r1[:], in1=bv1_row,
                op0=MUL, op1=ADD)
            ctx1_bf = small.tile([1, D], bf16, tag="ctx1b")
            nc.vector.tensor_copy(ctx1_bf[:], ctx1[:])

            # gamma1 = ctx1 . wg11 + bg1 ;  c21 = ctx1 . u2
            jrow = small.tile([1, D], f32, tag="jrow")
            g1g = small.tile([1, 1], f32, tag="g1g")
            nc.vector.scalar_tensor_tensor(
                out=jrow[:], in0=ctx1[:], scalar=1.0, in1=wg11_row,
                op0=MUL, op1=MUL, accum_out=g1g[:])
            c21 = small.tile([1, 1], f32, tag="c21")
            nc.vector.scalar_tensor_tensor(
                out=jrow[:], in0=ctx1[:], scalar=1.0, in1=u2_row,
                op0=MUL, op1=MUL, accum_out=c21[:])

            # broadcast gamma1, c21 across partitions (PE ones trick)
            g1c_ps = ps_sm.tile([P, 1], f32, tag="sm")
            nc.tensor.matmul(g1c_ps[:], ones_rf[:], g1g[:], start=True, stop=False)
            nc.tensor.matmul(g1c_ps[:], ones_rf[:], bgs[:, 0:1], start=False,
                             stop=True)
            g1col = small.tile([P, 1], f32, tag="g1col")
            nc.vector.tensor_copy(g1col[:], g1c_ps[:])
            c21c_ps = ps_sm.tile([P, 1], f32, tag="sm")
            nc.tensor.matmul(c21c_ps[:], ones_rf[:], c21[:], start=True, stop=True)
            c21col = small.tile([P, 1], f32, tag="c21col")
            nc.vector.tensor_copy(c21col[:], c21c_ps[:])

            # g1 = sigmoid(gx1 + gamma1)
            g1 = small.tile([P, T], f32)
            nc.scalar.activation(g1[:], gx1[:], SIG, bias=g1col[:])

            # g1 transposed to rows (for outer products), bf16, flattened to
            # one partition so row slices are PE-legal (base partition 0)
            g1t_ps = ps_sm.tile([T, P], f32, tag="sm")
            nc.tensor.transpose(g1t_ps[:], g1[:], eye[:])
            g1t_sb = small.tile([T, P], bf16, tag="g1ts")
            nc.vector.tensor_copy(g1t_sb[:], g1t_ps[:])
            g1t = small.tile([1, T * P], bf16, tag="g1t")
            nc.gpsimd.dma_start(g1t[:], g1t_sb[:])

            # ctx1 broadcast tile (f32) for the ACT-path output combine
            cb1_ps = ps_sm.tile([P, D], f32, tag="sm")
            nc.tensor.matmul(cb1_ps[:], ones_rb[:], ctx1_bf[:], start=True,
                             stop=True)
            ctx1_bc = big.tile([P, D], f32, tag="ctx1bc")
            nc.scalar.copy(ctx1_bc[:], cb1_ps[:])

            # =============== layer 2 attention (kv = x_new) ==============
            # kdot2 = 2*(x@u2) + c21*g1   (sx2 already includes the 2x fold)
            sk2 = small.tile([P, T], f32)
            nc.vector.scalar_tensor_tensor(
                out=sk2[:], in0=g1[:], scalar=c21col[:], in1=sx2[:],
                op0=MUL, op1=ADD)
            e2f = small.tile([P, T], f32)
            nc.scalar.activation(e2f[:], sk2[:], EXP)
            e2m = small.tile([P, T], f32)
            nc.vector.tensor_tensor(out=e2m[:], in0=e2f[:], in1=mask_sb[:],
                                    op=MUL)
            e2b = small.tile([P, T], bf16)   # 2*e2, bf16
            nc.vector.tensor_scalar(out=e2b[:], in0=e2m[:], scalar1=2.0,
                                    scalar2=None, op0=MUL)

            a2_ps = ps_sm.tile([1, T], f32, tag="sm")
            nc.tensor.matmul(a2_ps[:], ones_cf[:], e2m[:], start=True, stop=True)
            a2 = small.tile([1, 1], f32, tag="a2")
            nc.vector.tensor_reduce(a2[:], a2_ps[:], axis=mybir.AxisListType.X,
                                    op=ADD)
            r2 = small.tile([1, 1], f32, tag="r2")
            nc.vector.reciprocal(r2[:], a2[:])

            # dot22 = sum(e2 * g1) -> cross-partition sum
            jcol = small.tile([P, T], f32, tag="jcol")
            d22p = small.tile([P, 1], f32, tag="d22p")
            nc.vector.scalar_tensor_tensor(
                out=jcol[:], in0=e2m[:], scalar=1.0, in1=g1[:],
                op0=MUL, op1=MUL, accum_out=d22p[:])
            d22_ps = ps_sm.tile([1, 1], f32, tag="sm")
            nc.tensor.matmul(d22_ps[:], ones_cf[:], d22p[:], start=True,
                             stop=True)
            d22 = small.tile([1, 1], bf16, tag="d22")
            nc.vector.tensor_copy(d22[:], d22_ps[:])

            # xbarT2 = (2 e2) @ x + dot22 * ctx1   (unnormalized)
            xb2_ps = ps_xb.tile([P, 2], f32, tag="xb")
            for c in range(2):
                for t in range(T):
                    nc.tensor.matmul(
                        xb2_ps[:, c:c + 1],
                        x_bf[:, t, c * P:(c + 1) * P],
                        e2b[:, t:t + 1],
                        start=(t == 0), stop=False)
                nc.tensor.matmul(
                    xb2_ps[:, c:c + 1],
                    ctx1_bf[:, c * P:(c + 1) * P],
                    d22[:],
                    start=False, stop=True)
            xb2 = small.tile([P, 2], bf16, tag="xb2s")
            nc.vector.tensor_copy(xb2[:], xb2_ps[:])

            c2_ps = ps_sm.tile([1, D], f32, tag="sm")
            for c in range(2):
                nc.tensor.matmul(c2_ps[:], xb2[:, c:c + 1], wv2[:, c, :],
                                 start=(c == 0), stop=(c == 1))
            ctx2 = small.tile([1, D], f32, tag="ctx2")
            nc.vector.scalar_tensor_tensor(
                out=ctx2[:], in0=c2_ps[:], scalar=r2[:], in1=bv2_row,
                op0=MUL, op1=ADD)
            ctx2_bf = small.tile([1, D], bf16, tag="ctx2b")
            nc.vector.tensor_copy(ctx2_bf[:], ctx2[:])

            g2g = small.tile([1, 1], f32, tag="g2g")
            nc.vector.scalar_tensor_tensor(
                out=jrow[:], in0=ctx2[:], scalar=1.0, in1=wg12_row,
                op0=MUL, op1=MUL, accum_out=g2g[:])
            g2c_ps = ps_sm.tile([P, 1], f32, tag="sm")
            nc.tensor.matmul(g2c_ps[:], ones_rf[:], g2g[:], start=True, stop=False)
            nc.tensor.matmul(g2c_ps[:], ones_rf[:], bgs[:, 1:2], start=False,
                             stop=True)
            g2col = small.tile([P, 1], f32, tag="g2col")
            nc.vector.tensor_copy(g2col[:], g2c_ps[:])

            g2 = small.tile([P, T], f32)
            nc.scalar.activation(g2[:], gp2[:], SIG, bias=g2col[:])
            g2t_ps = ps_sm.tile([T, P], f32, tag="sm")
            nc.tensor.transpose(g2t_ps[:], g2[:], eye[:])
            g2t_sb = small.tile([T, P], bf16, tag="g2ts")
            nc.vector.tensor_copy(g2t_sb[:], g2t_ps[:])
            g2t = small.tile([1, T * P], bf16, tag="g2t")
            nc.gpsimd.dma_start(g2t[:], g2t_sb[:])

            cb2_ps = ps_sm.tile([P, D], f32, tag="sm")
            nc.tensor.matmul(cb2_ps[:], ones_rb[:], ctx2_bf[:], start=True,
                             stop=True)
            ctx2_bc = big.tile([P, D], f32, tag="ctx2bc")
            nc.scalar.copy(ctx2_bc[:], cb2_ps[:])

            # =============== output combine + stores =====================
            # x_new = 2x + g1 (x) ctx1 ;  p_new = 2p + g2 (x) ctx2
            for (src, dst, gt, gcols, cbf, cbc, out_d) in (
                (x_sb, xn_sb, g1t, g1, ctx1_bf, ctx1_bc, xo_d),
                (p_sb, pn_sb, g2t, g2, ctx2_bf, ctx2_bc, po_d),
            ):
                for t in range(T):
                    if t < NS:
                        # PE path: psum = 2I @ src + g^T (outer) ctx
                        gp = ps_g.tile([P, D], f32, tag="gps")
                        nc.tensor.matmul(gp[:], twoi[:], src[:, t, :],
                                         start=True, stop=False)
                        nc.tensor.matmul(gp[:], gt[0:1, t * P:(t + 1) * P],
                                         cbf[:], start=False, stop=True)
                        nc.scalar.copy(dst[:, t, :], gp[:])
                    else:
                        # ACT+DVE path: tmp = g*ctx_bc; dst = 2*src + tmp
                        tmp = big.tile([P, D], f32, tag="gtmp")
                        nc.scalar.activation(tmp[:], cbc[:], CPY,
                                             scale=gcols[:, t:t + 1])
                        nc.vector.scalar_tensor_tensor(
                            out=dst[:, t, :], in0=src[:, t, :], scalar=2.0,
                            in1=tmp[:], op0=MUL, op1=ADD)
                for ch in range(0, T, CHUNK):
                    s = slice(ch * D, (ch + CHUNK) * D)
                    nc.sync.dma_start(out_d[:, s], dst[:, ch:ch + CHUNK, :])

    nc.finalize()

    # ---- per-core inputs ------------------------------------------------
    eye_np = np.eye(P, dtype=np.float32)
    shared = {
        "wv1": np.ascontiguousarray(
            fold["ra1_Wv_bf"].reshape(2, P, D).transpose(1, 0, 2).reshape(P, 2 * D)),
        "wv2": np.ascontiguousarray(
            fold["ra2_Wv_bf"].reshape(2, P, D).transpose(1, 0, 2).reshape(P, 2 * D)),
        "eye": eye_np,
        "two_i": 2.0 * eye_np,
        "ones_r_f": np.ones((1, P), np.float32),
        "ones_r_b": np.ones((1, P), ml_dtypes.bfloat16),
        "ones_c_f": np.ones((P, 1), np.float32),
        "bgs": np.array([[fold["ra1_bg"], fold["ra2_bg"]]], np.float32),
        "rows_b": np.concatenate([
            fold["ra1_u"], 2.0 * fold["ra2_u"], fold["ra1_w"], fold["ra2_w"],
        ]).astype(ml_dtypes.bfloat16).reshape(1, 4 * D),
        "rows_f": np.concatenate([
            fold["ra2_u"], fold["ra1_wg1"], fold["ra2_wg1"],
            fold["ra1_bv"], fold["ra2_bv"],
        ]).astype(np.float32).reshape(1, 5 * D),
    }
    x_np = np.asarray(inputs["x"], dtype=np.float32)
    p_np = np.asarray(inputs["p"], dtype=np.float32)
    m_np = np.asarray(inputs["mask"]).astype(np.float32)
    in_maps = []
    for b in range(NCORES):
        im = dict(shared)
        im["x"] = _perm(x_np[b])
        im["p"] = _perm(p_np[b])
        im["mask"] = np.ascontiguousarray(m_np[b].reshape(T, P).T)
        in_maps.append(im)

    def post(results):
        x_new = np.stack([_unperm(results[b]["x_out"]) for b in range(NCORES)])
        p_new = np.stack([_unperm(results[b]["p_out"]) for b in range(NCORES)])
        return x_new, p_new

    return nc, in_maps, post


def kernel(**inputs):
    from concourse.bass_utils import run_bass_kernel_spmd

    nc, in_maps, post = build(inputs)
    res = run_bass_kernel_spmd(nc, in_maps, core_ids=list(range(NCORES)))
    return post(res.results)
